# revision 5
# baseline (speedup 1.0000x reference)
"""Causal self-attention (B=2, T=2048, C=1024, H=16) on 8 Trainium2 NeuronCores.

Sharding: data-parallel over batch (2) x tensor-parallel over heads (4 groups
of 4 heads) = 8 cores. c_attn column-sharded, c_proj row-sharded; each core
emits a partial [C, T] projection output that the host sums per batch.

All matmuls run in bf16 with fp32 PSUM accumulation. Attention scores are
computed transposed (S^T = K Q^T, k on partitions). The PV matmul keeps V
stationary (65 columns: 64 V dims + a ones column that accumulates the
softmax denominator) and streams P 512 wide, which minimizes the per-matmul
LDWEIGHTS tax — walrus emits a serial weight load for every matmul here, so
small stationary tiles and wide moving operands are what count. The
normalization 1/denom row is broadcast across partitions with a K=1 matmul,
so no transposes are needed anywhere: yT comes out of PV directly.
"""

import numpy as np
import ml_dtypes

BF = ml_dtypes.bfloat16

B, T, C, H, DH = 2, 2048, 1024, 16, 64
N_CORES = 8
G = 4            # head groups (tensor-parallel)
HPG = 4          # heads per group
TQ = 512         # query strip width
TK = 128         # key tile width
NSTRIP = T // TQ        # 4 query strips
NKT = T // TK           # 16 key tiles
NCT = C // 128          # 8 contraction tiles for qkv
VST = 136               # V2 per-k-tile stride: 2 heads x (64 V + 1 ones + 3 pad)

_CACHE = {}


def _ensure_runtime():
    """Import jax (boots the axon PJRT plugin) exactly once."""
    import jax
    jax.devices()


def _build(with_bias: bool):
    import concourse.tile as tile
    from concourse import bacc, mybir

    f32 = mybir.dt.float32
    bf16 = mybir.dt.bfloat16
    Exp = mybir.ActivationFunctionType.Exp

    nc = bacc.Bacc("TRN2", target_bir_lowering=False, debug=False,
                   enable_asserts=False, num_devices=N_CORES)

    xT_d = nc.dram_tensor("xT", [C, T], bf16, kind="ExternalInput").ap()
    wqk_d = nc.dram_tensor("wqk", [C, 512], bf16, kind="ExternalInput").ap()
    wv_d = nc.dram_tensor("wv", [C, 256], bf16, kind="ExternalInput").ap()
    wp_d = nc.dram_tensor("wp", [256, C], bf16, kind="ExternalInput").ap()
    if with_bias:
        bqk_d = nc.dram_tensor("bqk", [1, 512], bf16, kind="ExternalInput").ap()
        bv_d = nc.dram_tensor("bv", [1, 256], bf16, kind="ExternalInput").ap()
    out_d = nc.dram_tensor("outT", [C, T], f32, kind="ExternalOutput").ap()

    with tile.TileContext(nc) as tc:
        with (
            tc.tile_pool(name="persist", bufs=1) as pp,
            tc.tile_pool(name="pP", bufs=34) as pP,
            tc.tile_pool(name="rrow", bufs=3) as pRR,
            tc.tile_pool(name="bcsb", bufs=3) as pBC,
            tc.tile_pool(name="ob", bufs=4) as pO,
            tc.tile_pool(name="psum", bufs=1, space="PSUM") as psp,
        ):
            # ---- persistent SBUF tensors -------------------------------
            xT = [pp.tile([128, T], bf16, tag=f"xT{i}", name=f"xT{i}")
                  for i in range(NCT)]
            wqk = [pp.tile([128, 512], bf16, tag=f"wqk{i}", name=f"wqk{i}")
                   for i in range(NCT)]
            wv = [pp.tile([128, 256], bf16, tag=f"wv{i}", name=f"wv{i}")
                  for i in range(NCT)]
            wp = [pp.tile([128, C], bf16, tag=f"wp{p}", name=f"wp{p}")
                  for p in range(2)]
            QT2 = [pp.tile([128, T], bf16, tag=f"QT{h}", name=f"QT{h}")
                   for h in range(HPG)]
            KT2 = [pp.tile([128, T // 2], bf16, tag=f"KT{h}", name=f"KT{h}")
                   for h in range(HPG)]
            V2 = [pp.tile([128, NKT * VST], bf16, tag=f"V{p}", name=f"V{p}")
                  for p in range(2)]
            yT2 = [pp.tile([128, T], bf16, tag=f"yT{p}", name=f"yT{p}")
                   for p in range(2)]
            ones64 = pp.tile([1, 64], bf16, tag="ones64", name="ones64")
            if with_bias:
                bqk = pp.tile([1, 512], bf16, tag="bqk", name="bqk")
                bv = pp.tile([1, 256], bf16, tag="bv", name="bv")
                ones_row = pp.tile([1, 512], bf16, tag="ones", name="ones")

            # ---- input DMAs + constants --------------------------------
            for i in range(NCT):
                nc.sync.dma_start(wqk[i][:], wqk_d[i * 128:(i + 1) * 128, :])
                nc.sync.dma_start(xT[i][:], xT_d[i * 128:(i + 1) * 128, :])
                nc.sync.dma_start(wv[i][:], wv_d[i * 128:(i + 1) * 128, :])
            for p in range(2):
                nc.sync.dma_start(wp[p][:], wp_d[p * 128:(p + 1) * 128, :])
            nc.gpsimd.memset(ones64[:], 1.0)
            if with_bias:
                nc.sync.dma_start(bqk[:], bqk_d[:, :])
                nc.sync.dma_start(bv[:], bv_d[:, :])
                nc.gpsimd.memset(ones_row[:], 1.0)
            for p in range(2):
                for kt in range(NKT):
                    for hh in range(2):
                        nc.gpsimd.memset(
                            V2[p][:, kt * VST + hh * 68 + 64: kt * VST + hh * 68 + 65],
                            1.0)

            nbias = 1 if with_bias else 0
            P_store = {}

            def emit_A(pair, qt):
                """QKV projection chunk: q/k M-tiles for T-strip qt; pair-0
                chunks also produce V (all 4 heads, N=256) for their k-tiles."""
                for mt in (pair, 2 + pair):
                    ps = psp.tile([128, TQ], f32, tag="big", bufs=2, name="psA")
                    for ci in range(NCT):
                        nc.tensor.matmul(
                            ps[:],
                            lhsT=wqk[ci][:, mt * 128:(mt + 1) * 128],
                            rhs=xT[ci][:, qt * TQ:(qt + 1) * TQ],
                            start=(ci == 0), stop=(ci == NCT + nbias - 1))
                    if with_bias:
                        nc.tensor.matmul(
                            ps[:], lhsT=bqk[0:1, mt * 128:(mt + 1) * 128],
                            rhs=ones_row[0:1, 0:TQ], start=False, stop=True)
                    for hh in range(2):
                        h = 2 * pair + hh
                        src = ps[hh * 64:(hh + 1) * 64, :]
                        if mt < 2:      # Q: native half via DVE, mirror via DMA
                            nc.vector.tensor_copy(
                                QT2[h][hh * 64:(hh + 1) * 64, qt * TQ:(qt + 1) * TQ], src)
                            nc.sync.dma_start(
                                QT2[h][(1 - hh) * 64:(2 - hh) * 64, qt * TQ:(qt + 1) * TQ],
                                QT2[h][hh * 64:(hh + 1) * 64, qt * TQ:(qt + 1) * TQ])
                        else:           # K: even k-tiles -> top half, odd -> bottom
                            s3 = src.rearrange("p (a b) -> p a b", b=128)
                            d3 = KT2[h][:, 2 * qt * 128: (2 * qt + 2) * 128] \
                                .rearrange("p (a b) -> p a b", b=128)
                            nc.vector.tensor_copy(d3[0:64, :, :], s3[:, 0:4:2, :])
                            nc.vector.tensor_copy(d3[64:128, :, :], s3[:, 1:4:2, :])
                if pair == 0:
                    for kt in range(4 * qt, 4 * qt + 4):
                        psv = psp.tile([128, 256], f32, tag="big", bufs=2, name="psVt")
                        for ci in range(NCT):
                            nc.tensor.matmul(
                                psv[:],
                                lhsT=xT[ci][:, kt * 128:(kt + 1) * 128],
                                rhs=wv[ci][:, :],
                                start=(ci == 0), stop=(ci == NCT + nbias - 1))
                        if with_bias:
                            nc.tensor.matmul(
                                psv[:], lhsT=ones_row[0:1, 0:128], rhs=bv[0:1, :],
                                start=False, stop=True)
                        for p in range(2):
                            s3 = psv[:, p * 128:(p + 1) * 128] \
                                .rearrange("q (a b) -> q a b", b=64)
                            d3 = V2[p][:, kt * VST: kt * VST + VST] \
                                .rearrange("q (a b) -> q a b", b=68)[:, :, 0:64]
                            nc.vector.tensor_copy(d3, s3)

            def emit_S(pair, qt):
                """Scores + exp for both heads of `pair` over strip qt."""
                nk = 4 * (qt + 1)
                for hh in range(2):
                    h = 2 * pair + hh
                    for u in range(nk // 2):
                        pss = []
                        for half in range(2):
                            ps = psp.tile([128, TQ], f32, tag="S", bufs=3, name="psS")
                            nc.tensor.matmul(
                                ps[:],
                                lhsT=KT2[h][half * 64:(half + 1) * 64,
                                            u * 128:(u + 1) * 128],
                                rhs=QT2[h][half * 64:(half + 1) * 64,
                                           qt * TQ:(qt + 1) * TQ],
                                start=True, stop=True)
                            pss.append(ps)
                        for half in range(2):
                            kt = 2 * u + half
                            m = kt - 4 * qt
                            off = max(0, m) * 128
                            Pt = pP.tile([128, TQ], bf16, tag="P", name="Pt")
                            nc.scalar.activation(Pt[:, off:TQ], pss[half][:, off:TQ],
                                                 Exp, scale=0.125)
                            if m > 0:    # left columns are fully masked: zero them
                                nc.gpsimd.memset(Pt[:, 0:off], 0.0)
                            if m >= 0:   # diagonal block: keep k <= q only
                                nc.gpsimd.affine_select(
                                    out=Pt[:, off:off + 128], in_=Pt[:, off:off + 128],
                                    compare_op=mybir.AluOpType.is_ge, fill=0.0,
                                    base=0, pattern=[[1, 128]], channel_multiplier=-1)
                            P_store[(h, qt, kt)] = Pt

            def emit_PV(pair, qt):
                """PV with V stationary: yT (+denominator row) per head-strip,
                then broadcast-normalize straight into yT2."""
                nk = 4 * (qt + 1)
                for hh in range(2):
                    h = 2 * pair + hh
                    psy = psp.tile([65, TQ], f32, tag="pv", bufs=3, name="psy")
                    for kt in range(nk):
                        nc.tensor.matmul(
                            psy[:],
                            lhsT=V2[pair][:, kt * VST + hh * 68: kt * VST + hh * 68 + 65],
                            rhs=P_store[(h, qt, kt)][:],
                            start=(kt == 0), stop=(kt == nk - 1))
                    rrow = pRR.tile([1, TQ], bf16, tag="rr", name="rrow")
                    with nc.allow_low_precision("softmax recip row in bf16"):
                        nc.vector.reciprocal(rrow[:], psy[64:65, :])
                    bc = psp.tile([64, TQ], f32, tag="pv", bufs=3, name="psbc")
                    nc.tensor.matmul(bc[:], lhsT=ones64[:], rhs=rrow[:],
                                     start=True, stop=True)
                    bcs = pBC.tile([64, TQ], bf16, tag="bc", name="bcs")
                    nc.vector.tensor_copy(bcs[:], bc[:])
                    nc.vector.tensor_mul(
                        yT2[pair][hh * 64:(hh + 1) * 64, qt * TQ:(qt + 1) * TQ],
                        psy[0:64, :], bcs[:])
                for kt in range(nk):
                    for hh in range(2):
                        del P_store[(2 * pair + hh, qt, kt)]

            def emit_PJ(qt):
                """Output projection for strip qt (both pairs)."""
                for co in range(8):
                    pso = psp.tile([128, TQ], f32, tag="big", bufs=2, name="psO")
                    for pair in range(2):
                        nc.tensor.matmul(
                            pso[:],
                            lhsT=wp[pair][:, co * 128:(co + 1) * 128],
                            rhs=yT2[pair][:, qt * TQ:(qt + 1) * TQ],
                            start=(pair == 0), stop=(pair == 1))
                    ob = pO.tile([128, TQ], f32, tag="ob", name="ob")
                    nc.any.tensor_copy(ob[:], pso[:])
                    nc.sync.dma_start(
                        out_d[co * 128:(co + 1) * 128, qt * TQ:(qt + 1) * TQ], ob[:])

            # ---- software-pipelined emission order ---------------------
            emit_A(0, 0)
            emit_S(0, 0)
            emit_A(0, 1)
            emit_PV(0, 0)
            emit_A(1, 0)
            emit_S(0, 1)
            emit_A(0, 2)
            emit_PV(0, 1)
            emit_A(1, 1)
            emit_S(1, 0)
            emit_A(0, 3)
            emit_PV(1, 0)
            emit_PJ(0)
            emit_S(0, 2)
            emit_A(1, 2)
            emit_PV(0, 2)
            emit_S(1, 1)
            emit_PV(1, 1)
            emit_PJ(1)
            emit_S(0, 3)
            emit_A(1, 3)
            emit_PV(0, 3)
            emit_S(1, 2)
            emit_PV(1, 2)
            emit_PJ(2)
            emit_S(1, 3)
            emit_PV(1, 3)
            emit_PJ(3)

    nc.compile()
    return nc


def _get_nc(with_bias: bool):
    key = ("nc", with_bias)
    if key not in _CACHE:
        _ensure_runtime()
        _CACHE[key] = _build(with_bias)
    return _CACHE[key]


def _shard_inputs(x, w_qkv, b_qkv, w_proj, with_bias):
    """Build the 8 per-core input maps (bf16)."""
    in_maps = []
    for core in range(N_CORES):
        b, g = core // G, core % G
        hs = [g * HPG + i for i in range(HPG)]
        q_cols = [w_qkv[:, h * DH:(h + 1) * DH] for h in hs]
        k_cols = [w_qkv[:, C + h * DH: C + (h + 1) * DH] for h in hs]
        v_cols = [w_qkv[:, 2 * C + h * DH: 2 * C + (h + 1) * DH] for h in hs]
        m = {
            "xT": np.ascontiguousarray(x[b].T).astype(BF),
            "wqk": np.concatenate(q_cols + k_cols, axis=1).astype(BF),
            "wv": np.concatenate(v_cols, axis=1).astype(BF),
            "wp": np.concatenate(
                [w_proj[h * DH:(h + 1) * DH, :] for h in hs], axis=0).astype(BF),
        }
        if with_bias:
            bq = [b_qkv[h * DH:(h + 1) * DH] for h in hs]
            bk = [b_qkv[C + h * DH: C + (h + 1) * DH] for h in hs]
            bvs = [b_qkv[2 * C + h * DH: 2 * C + (h + 1) * DH] for h in hs]
            m["bqk"] = np.concatenate(bq + bk)[None, :].astype(BF)
            m["bv"] = np.concatenate(bvs)[None, :].astype(BF)
        in_maps.append(m)
    return in_maps


def run_on_device(x, w_qkv, b_qkv, w_proj, b_proj, trace=False, trace_kwargs=None):
    """Returns (output [B,T,C] float32, BassKernelResults)."""
    x = np.asarray(x, np.float32)
    w_qkv = np.asarray(w_qkv, np.float32)
    b_qkv = np.asarray(b_qkv, np.float32)
    w_proj = np.asarray(w_proj, np.float32)
    b_proj = np.asarray(b_proj, np.float32)

    with_bias = bool(np.any(b_qkv))
    nc = _get_nc(with_bias)
    in_maps = _shard_inputs(x, w_qkv, b_qkv, w_proj, with_bias)

    from concourse.bass_utils import run_bass_kernel_spmd
    res = run_bass_kernel_spmd(nc, in_maps, core_ids=list(range(N_CORES)),
                               trace=trace, **(trace_kwargs or {}))

    out = np.zeros((B, T, C), np.float64)
    for core in range(N_CORES):
        b = core // G
        out[b] += res.results[core]["outT"].T.astype(np.float64)
    out += b_proj.astype(np.float64)[None, None, :]
    return out.astype(np.float32), res


def kernel(x, w_qkv, b_qkv, w_proj, b_proj):
    out, _ = run_on_device(x, w_qkv, b_qkv, w_proj, b_proj)
    return out


# revision 6
# speedup vs baseline: 1.1086x; 1.1086x over previous
"""Causal self-attention (B=2, T=2048, C=1024, H=16) on 8 Trainium2 NeuronCores.

Sharding: data-parallel over batch (2) x tensor-parallel over heads (4 groups
of 4 heads) = 8 cores. c_attn column-sharded, c_proj row-sharded; each core
emits a partial [C, T] projection output that the host sums per batch.

All matmuls run in bf16 with fp32 PSUM accumulation. Attention scores are
computed transposed (S^T = K Q^T, k on partitions). The PV matmul keeps V
stationary (65 columns: 64 V dims + a ones column that accumulates the
softmax denominator) and streams P 512 wide, which minimizes the per-matmul
LDWEIGHTS tax — walrus emits a serial weight load for every matmul here, so
small stationary tiles and wide moving operands are what count. The
normalization 1/denom row is broadcast across partitions with a K=1 matmul,
so no transposes are needed anywhere: yT comes out of PV directly.
"""

import numpy as np
import ml_dtypes

BF = ml_dtypes.bfloat16

B, T, C, H, DH = 2, 2048, 1024, 16, 64
N_CORES = 8
G = 4            # head groups (tensor-parallel)
HPG = 4          # heads per group
TQ = 512         # query strip width
TK = 128         # key tile width
NSTRIP = T // TQ        # 4 query strips
NKT = T // TK           # 16 key tiles
NCT = C // 128          # 8 contraction tiles for qkv
VST = 136               # V2 per-k-tile stride: 2 heads x (64 V + 1 ones + 3 pad)

_CACHE = {}


def _ensure_runtime():
    """Import jax (boots the axon PJRT plugin) exactly once."""
    import jax
    jax.devices()


def _build(with_bias: bool):
    import concourse.tile as tile
    from concourse import bacc, mybir

    f32 = mybir.dt.float32
    bf16 = mybir.dt.bfloat16
    Exp = mybir.ActivationFunctionType.Exp

    nc = bacc.Bacc("TRN2", target_bir_lowering=False, debug=False,
                   enable_asserts=False, num_devices=N_CORES)

    xT_d = nc.dram_tensor("xT", [C, T], bf16, kind="ExternalInput").ap()
    wqk_d = nc.dram_tensor("wqk", [C, 512], bf16, kind="ExternalInput").ap()
    wv_d = nc.dram_tensor("wv", [C, 256], bf16, kind="ExternalInput").ap()
    wp_d = nc.dram_tensor("wp", [256, C], bf16, kind="ExternalInput").ap()
    if with_bias:
        bqk_d = nc.dram_tensor("bqk", [1, 512], bf16, kind="ExternalInput").ap()
        bv_d = nc.dram_tensor("bv", [1, 256], bf16, kind="ExternalInput").ap()
    out_d = nc.dram_tensor("outT", [C, T], f32, kind="ExternalOutput").ap()

    with tile.TileContext(nc) as tc:
        with (
            tc.tile_pool(name="persist", bufs=1) as pp,
            tc.tile_pool(name="pP", bufs=34) as pP,
            tc.tile_pool(name="rrow", bufs=3) as pRR,
            tc.tile_pool(name="bcsb", bufs=3) as pBC,
            tc.tile_pool(name="ob", bufs=4) as pO,
            tc.tile_pool(name="psum", bufs=1, space="PSUM") as psp,
        ):
            # ---- persistent SBUF tensors -------------------------------
            xT = [pp.tile([128, T], bf16, tag=f"xT{i}", name=f"xT{i}")
                  for i in range(NCT)]
            wqk = [pp.tile([128, 512], bf16, tag=f"wqk{i}", name=f"wqk{i}")
                   for i in range(NCT)]
            wv = [pp.tile([128, 256], bf16, tag=f"wv{i}", name=f"wv{i}")
                  for i in range(NCT)]
            wp = [pp.tile([128, C], bf16, tag=f"wp{p}", name=f"wp{p}")
                  for p in range(2)]
            QT2 = [pp.tile([128, T], bf16, tag=f"QT{h}", name=f"QT{h}")
                   for h in range(HPG)]
            KT2 = [pp.tile([128, T // 2], bf16, tag=f"KT{h}", name=f"KT{h}")
                   for h in range(HPG)]
            V2 = [pp.tile([128, NKT * VST], bf16, tag=f"V{p}", name=f"V{p}")
                  for p in range(2)]
            yT2 = [pp.tile([128, T], bf16, tag=f"yT{p}", name=f"yT{p}")
                   for p in range(2)]
            ones64 = pp.tile([1, 64], bf16, tag="ones64", name="ones64")
            if with_bias:
                bqk = pp.tile([1, 512], bf16, tag="bqk", name="bqk")
                bv = pp.tile([1, 256], bf16, tag="bv", name="bv")
                ones_row = pp.tile([1, 512], bf16, tag="ones", name="ones")

            # ---- input DMAs + constants --------------------------------
            for i in range(NCT):
                nc.sync.dma_start(wqk[i][:], wqk_d[i * 128:(i + 1) * 128, :])
                nc.sync.dma_start(xT[i][:], xT_d[i * 128:(i + 1) * 128, :])
                nc.sync.dma_start(wv[i][:], wv_d[i * 128:(i + 1) * 128, :])
            for p in range(2):
                nc.sync.dma_start(wp[p][:], wp_d[p * 128:(p + 1) * 128, :])
            nc.gpsimd.memset(ones64[:], 1.0)
            if with_bias:
                nc.sync.dma_start(bqk[:], bqk_d[:, :])
                nc.sync.dma_start(bv[:], bv_d[:, :])
                nc.gpsimd.memset(ones_row[:], 1.0)
            for p in range(2):
                for kt in range(NKT):
                    for hh in range(2):
                        nc.gpsimd.memset(
                            V2[p][:, kt * VST + hh * 68 + 64: kt * VST + hh * 68 + 65],
                            1.0)

            nbias = 1 if with_bias else 0
            P_store = {}

            def emit_A(pair, qt):
                """QKV projection chunk: q/k M-tiles for T-strip qt; pair-0
                chunks also produce V (all 4 heads, N=256) for their k-tiles."""
                for mt in (pair, 2 + pair):
                    ps = psp.tile([128, TQ], f32, tag="big", bufs=2, name="psA")
                    for ci in range(NCT):
                        nc.tensor.matmul(
                            ps[:],
                            lhsT=wqk[ci][:, mt * 128:(mt + 1) * 128],
                            rhs=xT[ci][:, qt * TQ:(qt + 1) * TQ],
                            start=(ci == 0), stop=(ci == NCT + nbias - 1))
                    if with_bias:
                        nc.tensor.matmul(
                            ps[:], lhsT=bqk[0:1, mt * 128:(mt + 1) * 128],
                            rhs=ones_row[0:1, 0:TQ], start=False, stop=True)
                    for hh in range(2):
                        h = 2 * pair + hh
                        src = ps[hh * 64:(hh + 1) * 64, :]
                        if mt < 2:      # Q: native half via DVE, mirror via DMA
                            nc.vector.tensor_copy(
                                QT2[h][hh * 64:(hh + 1) * 64, qt * TQ:(qt + 1) * TQ], src)
                            nc.sync.dma_start(
                                QT2[h][(1 - hh) * 64:(2 - hh) * 64, qt * TQ:(qt + 1) * TQ],
                                QT2[h][hh * 64:(hh + 1) * 64, qt * TQ:(qt + 1) * TQ])
                        else:           # K: even k-tiles -> top half, odd -> bottom
                            s3 = src.rearrange("p (a b) -> p a b", b=128)
                            d3 = KT2[h][:, 2 * qt * 128: (2 * qt + 2) * 128] \
                                .rearrange("p (a b) -> p a b", b=128)
                            nc.vector.tensor_copy(d3[0:64, :, :], s3[:, 0:4:2, :])
                            nc.vector.tensor_copy(d3[64:128, :, :], s3[:, 1:4:2, :])
                if pair == 0:
                    for kt in range(4 * qt, 4 * qt + 4):
                        psv = psp.tile([128, 256], f32, tag="big", bufs=2, name="psVt")
                        for ci in range(NCT):
                            nc.tensor.matmul(
                                psv[:],
                                lhsT=xT[ci][:, kt * 128:(kt + 1) * 128],
                                rhs=wv[ci][:, :],
                                start=(ci == 0), stop=(ci == NCT + nbias - 1))
                        if with_bias:
                            nc.tensor.matmul(
                                psv[:], lhsT=ones_row[0:1, 0:128], rhs=bv[0:1, :],
                                start=False, stop=True)
                        for p in range(2):
                            s3 = psv[:, p * 128:(p + 1) * 128] \
                                .rearrange("q (a b) -> q a b", b=64)
                            d3 = V2[p][:, kt * VST: kt * VST + VST] \
                                .rearrange("q (a b) -> q a b", b=68)[:, :, 0:64]
                            nc.vector.tensor_copy(d3, s3)

            def emit_S(pair, qt):
                """Scores + exp for both heads of `pair` over strip qt."""
                nk = 4 * (qt + 1)
                for hh in range(2):
                    h = 2 * pair + hh
                    for u in range(nk // 2):
                        pss = []
                        for half in range(2):
                            ps = psp.tile([128, TQ], f32, tag="S", bufs=3, name="psS")
                            nc.tensor.matmul(
                                ps[:],
                                lhsT=KT2[h][half * 64:(half + 1) * 64,
                                            u * 128:(u + 1) * 128],
                                rhs=QT2[h][half * 64:(half + 1) * 64,
                                           qt * TQ:(qt + 1) * TQ],
                                start=True, stop=True)
                            pss.append(ps)
                        for half in range(2):
                            kt = 2 * u + half
                            m = kt - 4 * qt
                            off = max(0, m) * 128
                            Pt = pP.tile([128, TQ], bf16, tag="P", name="Pt")
                            nc.scalar.activation(Pt[:, off:TQ], pss[half][:, off:TQ],
                                                 Exp, scale=0.125)
                            if m > 0:    # left columns are fully masked: zero them
                                nc.gpsimd.memset(Pt[:, 0:off], 0.0)
                            if m >= 0:   # diagonal block: keep k <= q only
                                nc.gpsimd.affine_select(
                                    out=Pt[:, off:off + 128], in_=Pt[:, off:off + 128],
                                    compare_op=mybir.AluOpType.is_ge, fill=0.0,
                                    base=0, pattern=[[1, 128]], channel_multiplier=-1)
                            P_store[(h, qt, kt)] = Pt

            def emit_PV(pair, qt):
                """PV with V stationary: yT (+denominator row) per head-strip,
                then broadcast-normalize straight into yT2."""
                nk = 4 * (qt + 1)
                for hh in range(2):
                    h = 2 * pair + hh
                    psy = psp.tile([65, TQ], f32, tag="pv", bufs=3, name="psy")
                    for kt in range(nk):
                        nc.tensor.matmul(
                            psy[:],
                            lhsT=V2[pair][:, kt * VST + hh * 68: kt * VST + hh * 68 + 65],
                            rhs=P_store[(h, qt, kt)][:],
                            start=(kt == 0), stop=(kt == nk - 1))
                    drow = pRR.tile([1, TQ], bf16, tag="rr", name="drow")
                    with nc.allow_low_precision("softmax denom row in bf16"):
                        nc.vector.tensor_copy(drow[:], psy[64:65, :])
                    bc = psp.tile([64, TQ], f32, tag="pv", bufs=3, name="psbc")
                    nc.tensor.matmul(bc[:], lhsT=ones64[:], rhs=drow[:],
                                     start=True, stop=True)
                    bcs = pBC.tile([64, TQ], bf16, tag="bc", name="bcs")
                    with nc.allow_low_precision("softmax recip in bf16"):
                        nc.vector.reciprocal(bcs[:], bc[:])
                    nc.vector.tensor_mul(
                        yT2[pair][hh * 64:(hh + 1) * 64, qt * TQ:(qt + 1) * TQ],
                        psy[0:64, :], bcs[:])
                for kt in range(nk):
                    for hh in range(2):
                        del P_store[(2 * pair + hh, qt, kt)]

            def emit_PJ(qt):
                """Output projection for strip qt (both pairs)."""
                for co in range(8):
                    pso = psp.tile([128, TQ], f32, tag="big", bufs=2, name="psO")
                    for pair in range(2):
                        nc.tensor.matmul(
                            pso[:],
                            lhsT=wp[pair][:, co * 128:(co + 1) * 128],
                            rhs=yT2[pair][:, qt * TQ:(qt + 1) * TQ],
                            start=(pair == 0), stop=(pair == 1))
                    ob = pO.tile([128, TQ], f32, tag="ob", name="ob")
                    nc.any.tensor_copy(ob[:], pso[:])
                    nc.sync.dma_start(
                        out_d[co * 128:(co + 1) * 128, qt * TQ:(qt + 1) * TQ], ob[:])

            # ---- software-pipelined emission order ---------------------
            emit_A(0, 0)
            emit_S(0, 0)
            emit_A(0, 1)
            emit_PV(0, 0)
            emit_A(1, 0)
            emit_S(0, 1)
            emit_A(0, 2)
            emit_PV(0, 1)
            emit_A(1, 1)
            emit_S(1, 0)
            emit_A(0, 3)
            emit_PV(1, 0)
            emit_PJ(0)
            emit_S(0, 2)
            emit_A(1, 2)
            emit_PV(0, 2)
            emit_S(1, 1)
            emit_PV(1, 1)
            emit_PJ(1)
            emit_S(0, 3)
            emit_A(1, 3)
            emit_PV(0, 3)
            emit_S(1, 2)
            emit_PV(1, 2)
            emit_PJ(2)
            emit_S(1, 3)
            emit_PV(1, 3)
            emit_PJ(3)

    nc.compile()
    return nc


def _get_nc(with_bias: bool):
    key = ("nc", with_bias)
    if key not in _CACHE:
        _ensure_runtime()
        _CACHE[key] = _build(with_bias)
    return _CACHE[key]


def _shard_inputs(x, w_qkv, b_qkv, w_proj, with_bias):
    """Build the 8 per-core input maps (bf16)."""
    in_maps = []
    for core in range(N_CORES):
        b, g = core // G, core % G
        hs = [g * HPG + i for i in range(HPG)]
        q_cols = [w_qkv[:, h * DH:(h + 1) * DH] for h in hs]
        k_cols = [w_qkv[:, C + h * DH: C + (h + 1) * DH] for h in hs]
        v_cols = [w_qkv[:, 2 * C + h * DH: 2 * C + (h + 1) * DH] for h in hs]
        m = {
            "xT": np.ascontiguousarray(x[b].T).astype(BF),
            "wqk": np.concatenate(q_cols + k_cols, axis=1).astype(BF),
            "wv": np.concatenate(v_cols, axis=1).astype(BF),
            "wp": np.concatenate(
                [w_proj[h * DH:(h + 1) * DH, :] for h in hs], axis=0).astype(BF),
        }
        if with_bias:
            bq = [b_qkv[h * DH:(h + 1) * DH] for h in hs]
            bk = [b_qkv[C + h * DH: C + (h + 1) * DH] for h in hs]
            bvs = [b_qkv[2 * C + h * DH: 2 * C + (h + 1) * DH] for h in hs]
            m["bqk"] = np.concatenate(bq + bk)[None, :].astype(BF)
            m["bv"] = np.concatenate(bvs)[None, :].astype(BF)
        in_maps.append(m)
    return in_maps


def run_on_device(x, w_qkv, b_qkv, w_proj, b_proj, trace=False, trace_kwargs=None):
    """Returns (output [B,T,C] float32, BassKernelResults)."""
    x = np.asarray(x, np.float32)
    w_qkv = np.asarray(w_qkv, np.float32)
    b_qkv = np.asarray(b_qkv, np.float32)
    w_proj = np.asarray(w_proj, np.float32)
    b_proj = np.asarray(b_proj, np.float32)

    with_bias = bool(np.any(b_qkv))
    nc = _get_nc(with_bias)
    in_maps = _shard_inputs(x, w_qkv, b_qkv, w_proj, with_bias)

    from concourse.bass_utils import run_bass_kernel_spmd
    res = run_bass_kernel_spmd(nc, in_maps, core_ids=list(range(N_CORES)),
                               trace=trace, **(trace_kwargs or {}))

    out = np.zeros((B, T, C), np.float64)
    for core in range(N_CORES):
        b = core // G
        out[b] += res.results[core]["outT"].T.astype(np.float64)
    out += b_proj.astype(np.float64)[None, None, :]
    return out.astype(np.float32), res


def kernel(x, w_qkv, b_qkv, w_proj, b_proj):
    out, _ = run_on_device(x, w_qkv, b_qkv, w_proj, b_proj)
    return out


# revision 7
# speedup vs baseline: 1.2048x; 1.0868x over previous
"""Causal self-attention (B=2, T=2048, C=1024, H=16) on 8 Trainium2 NeuronCores.

Sharding: data-parallel over batch (2) x tensor-parallel over heads (4 groups
of 4 heads) = 8 cores. c_attn column-sharded, c_proj row-sharded; each core
emits a partial [C, T] projection output that the host sums per batch.

All matmuls run in bf16 with fp32 PSUM accumulation. Attention scores are
computed transposed (S^T = K Q^T, k on partitions). The PV matmul keeps V
stationary (65 columns: 64 V dims + a ones column that accumulates the
softmax denominator) and streams P 512 wide, which minimizes the per-matmul
LDWEIGHTS tax — walrus emits a serial weight load for every matmul here, so
small stationary tiles and wide moving operands are what count. The
normalization 1/denom row is broadcast across partitions with a K=1 matmul,
so no transposes are needed anywhere: yT comes out of PV directly.
"""

import numpy as np
import ml_dtypes

BF = ml_dtypes.bfloat16

B, T, C, H, DH = 2, 2048, 1024, 16, 64
N_CORES = 8
G = 4            # head groups (tensor-parallel)
HPG = 4          # heads per group
TQ = 512         # query strip width
TK = 128         # key tile width
NSTRIP = T // TQ        # 4 query strips
NKT = T // TK           # 16 key tiles
NCT = C // 128          # 8 contraction tiles for qkv
VST = 136               # V2 per-k-tile stride: 2 heads x (64 V + 1 ones + 3 pad)

_CACHE = {}


def _ensure_runtime():
    """Import jax (boots the axon PJRT plugin) exactly once."""
    import jax
    jax.devices()


def _build(with_bias: bool):
    import concourse.tile as tile
    from concourse import bacc, mybir

    f32 = mybir.dt.float32
    bf16 = mybir.dt.bfloat16
    Exp = mybir.ActivationFunctionType.Exp

    nc = bacc.Bacc("TRN2", target_bir_lowering=False, debug=False,
                   enable_asserts=False, num_devices=N_CORES)

    xT_d = nc.dram_tensor("xT", [C, T], bf16, kind="ExternalInput").ap()
    wqk_d = nc.dram_tensor("wqk", [C, 512], bf16, kind="ExternalInput").ap()
    wv_d = nc.dram_tensor("wv", [C, 256], bf16, kind="ExternalInput").ap()
    wp_d = nc.dram_tensor("wp", [256, C], bf16, kind="ExternalInput").ap()
    if with_bias:
        bqk_d = nc.dram_tensor("bqk", [1, 512], bf16, kind="ExternalInput").ap()
        bv_d = nc.dram_tensor("bv", [1, 256], bf16, kind="ExternalInput").ap()
    out_d = nc.dram_tensor("outT", [C, T], f32, kind="ExternalOutput").ap()

    with tile.TileContext(nc) as tc:
        with (
            tc.tile_pool(name="persist", bufs=1) as pp,
            tc.tile_pool(name="pP", bufs=34) as pP,
            tc.tile_pool(name="rrow", bufs=3) as pRR,
            tc.tile_pool(name="bcsb", bufs=3) as pBC,
            tc.tile_pool(name="ob", bufs=4) as pO,
            tc.tile_pool(name="psum", bufs=1, space="PSUM") as psp,
        ):
            # ---- persistent SBUF tensors -------------------------------
            xT = [pp.tile([128, T], bf16, tag=f"xT{i}", name=f"xT{i}")
                  for i in range(NCT)]
            wqk = [pp.tile([128, 512], bf16, tag=f"wqk{i}", name=f"wqk{i}")
                   for i in range(NCT)]
            wv = [pp.tile([128, 256], bf16, tag=f"wv{i}", name=f"wv{i}")
                  for i in range(NCT)]
            wp = [pp.tile([128, C], bf16, tag=f"wp{p}", name=f"wp{p}")
                  for p in range(2)]
            QTp = [pp.tile([128, T], bf16, tag=f"QT{p}", name=f"QT{p}")
                   for p in range(2)]
            KTp = [pp.tile([128, T], bf16, tag=f"KT{p}", name=f"KT{p}")
                   for p in range(2)]
            V2 = [pp.tile([128, NKT * VST], bf16, tag=f"V{p}", name=f"V{p}")
                  for p in range(2)]
            yT2 = [pp.tile([128, T], bf16, tag=f"yT{p}", name=f"yT{p}")
                   for p in range(2)]
            ones64 = pp.tile([1, 64], bf16, tag="ones64", name="ones64")
            if with_bias:
                bqk = pp.tile([1, 512], bf16, tag="bqk", name="bqk")
                bv = pp.tile([1, 256], bf16, tag="bv", name="bv")
                ones_row = pp.tile([1, 512], bf16, tag="ones", name="ones")

            # ---- input DMAs + constants --------------------------------
            for i in range(NCT):
                nc.sync.dma_start(wqk[i][:], wqk_d[i * 128:(i + 1) * 128, :])
                nc.sync.dma_start(xT[i][:], xT_d[i * 128:(i + 1) * 128, :])
                nc.sync.dma_start(wv[i][:], wv_d[i * 128:(i + 1) * 128, :])
            for p in range(2):
                nc.sync.dma_start(wp[p][:], wp_d[p * 128:(p + 1) * 128, :])
            nc.gpsimd.memset(ones64[:], 1.0)
            if with_bias:
                nc.sync.dma_start(bqk[:], bqk_d[:, :])
                nc.sync.dma_start(bv[:], bv_d[:, :])
                nc.gpsimd.memset(ones_row[:], 1.0)
            for p in range(2):
                for kt in range(NKT):
                    for hh in range(2):
                        nc.gpsimd.memset(
                            V2[p][:, kt * VST + hh * 68 + 64: kt * VST + hh * 68 + 65],
                            1.0)

            nbias = 1 if with_bias else 0
            P_store = {}

            def emit_A(pair, qt):
                """QKV projection chunk: q/k M-tiles for T-strip qt; pair-0
                chunks also produce V (all 4 heads, N=256) for their k-tiles."""
                for mt in (pair, 2 + pair):
                    ps = psp.tile([128, TQ], f32, tag="big", bufs=2, name="psA")
                    for ci in range(NCT):
                        nc.tensor.matmul(
                            ps[:],
                            lhsT=wqk[ci][:, mt * 128:(mt + 1) * 128],
                            rhs=xT[ci][:, qt * TQ:(qt + 1) * TQ],
                            start=(ci == 0), stop=(ci == NCT + nbias - 1))
                    if with_bias:
                        nc.tensor.matmul(
                            ps[:], lhsT=bqk[0:1, mt * 128:(mt + 1) * 128],
                            rhs=ones_row[0:1, 0:TQ], start=False, stop=True)
                    dst = QTp[pair] if mt < 2 else KTp[pair]
                    nc.vector.tensor_copy(dst[:, qt * TQ:(qt + 1) * TQ], ps[:])
                if pair == 0:
                    for kt in range(4 * qt, 4 * qt + 4):
                        psv = psp.tile([128, 256], f32, tag="big", bufs=2, name="psVt")
                        for ci in range(NCT):
                            nc.tensor.matmul(
                                psv[:],
                                lhsT=xT[ci][:, kt * 128:(kt + 1) * 128],
                                rhs=wv[ci][:, :],
                                start=(ci == 0), stop=(ci == NCT + nbias - 1))
                        if with_bias:
                            nc.tensor.matmul(
                                psv[:], lhsT=ones_row[0:1, 0:128], rhs=bv[0:1, :],
                                start=False, stop=True)
                        for p in range(2):
                            s3 = psv[:, p * 128:(p + 1) * 128] \
                                .rearrange("q (a b) -> q a b", b=64)
                            d3 = V2[p][:, kt * VST: kt * VST + VST] \
                                .rearrange("q (a b) -> q a b", b=68)[:, :, 0:64]
                            nc.vector.tensor_copy(d3, s3)

            def emit_S(pair, qt):
                """Scores + exp for both heads of `pair` over strip qt.
                The two heads run concurrently on disjoint PE row groups."""
                nk = 4 * (qt + 1)
                for kt in range(nk):
                    pss = []
                    for hh in range(2):
                        ps = psp.tile([128, TQ], f32, tag="S", bufs=3, name="psS")
                        nc.tensor.matmul(
                            ps[:],
                            lhsT=KTp[pair][hh * 64:(hh + 1) * 64,
                                           kt * 128:(kt + 1) * 128],
                            rhs=QTp[pair][hh * 64:(hh + 1) * 64,
                                          qt * TQ:(qt + 1) * TQ],
                            start=True, stop=True)
                        pss.append(ps)
                    m = kt - 4 * qt
                    off = max(0, m) * 128
                    for hh in range(2):
                        h = 2 * pair + hh
                        Pt = pP.tile([128, TQ], bf16, tag="P", name="Pt")
                        nc.scalar.activation(Pt[:, off:TQ], pss[hh][:, off:TQ],
                                             Exp, scale=0.125)
                        if m > 0:    # left columns are fully masked: zero them
                            nc.gpsimd.memset(Pt[:, 0:off], 0.0)
                        if m >= 0:   # diagonal block: keep k <= q only
                            nc.gpsimd.affine_select(
                                out=Pt[:, off:off + 128], in_=Pt[:, off:off + 128],
                                compare_op=mybir.AluOpType.is_ge, fill=0.0,
                                base=0, pattern=[[1, 128]], channel_multiplier=-1)
                        P_store[(h, qt, kt)] = Pt

            def emit_PV(pair, qt):
                """PV with V stationary: yT (+denominator row) per head-strip,
                then broadcast-normalize straight into yT2."""
                nk = 4 * (qt + 1)
                for hh in range(2):
                    h = 2 * pair + hh
                    psy = psp.tile([65, TQ], f32, tag="pv", bufs=3, name="psy")
                    for kt in range(nk):
                        off = max(0, kt - 4 * qt) * 128
                        nc.tensor.matmul(
                            psy[:, off:TQ],
                            lhsT=V2[pair][:, kt * VST + hh * 68: kt * VST + hh * 68 + 65],
                            rhs=P_store[(h, qt, kt)][:, off:TQ],
                            start=(kt == 0), stop=(kt == nk - 1))
                    drow = pRR.tile([1, TQ], bf16, tag="rr", name="drow")
                    with nc.allow_low_precision("softmax denom row in bf16"):
                        nc.vector.tensor_copy(drow[:], psy[64:65, :])
                    bc = psp.tile([64, TQ], f32, tag="pv", bufs=3, name="psbc")
                    nc.tensor.matmul(bc[:], lhsT=ones64[:], rhs=drow[:],
                                     start=True, stop=True)
                    bcs = pBC.tile([64, TQ], bf16, tag="bc", name="bcs")
                    with nc.allow_low_precision("softmax recip in bf16"):
                        nc.vector.reciprocal(bcs[:], bc[:])
                    nc.vector.tensor_mul(
                        yT2[pair][hh * 64:(hh + 1) * 64, qt * TQ:(qt + 1) * TQ],
                        psy[0:64, :], bcs[:])
                for kt in range(nk):
                    for hh in range(2):
                        del P_store[(2 * pair + hh, qt, kt)]

            def emit_PJ(qt):
                """Output projection for strip qt (both pairs)."""
                for co in range(8):
                    pso = psp.tile([128, TQ], f32, tag="big", bufs=2, name="psO")
                    for pair in range(2):
                        nc.tensor.matmul(
                            pso[:],
                            lhsT=wp[pair][:, co * 128:(co + 1) * 128],
                            rhs=yT2[pair][:, qt * TQ:(qt + 1) * TQ],
                            start=(pair == 0), stop=(pair == 1))
                    ob = pO.tile([128, TQ], f32, tag="ob", name="ob")
                    nc.any.tensor_copy(ob[:], pso[:])
                    nc.sync.dma_start(
                        out_d[co * 128:(co + 1) * 128, qt * TQ:(qt + 1) * TQ], ob[:])

            # ---- software-pipelined emission order ---------------------
            emit_A(0, 0)
            emit_S(0, 0)
            emit_A(0, 1)
            emit_A(1, 0)
            emit_PV(0, 0)
            emit_S(0, 1)
            emit_A(0, 2)
            emit_A(1, 1)
            emit_PV(0, 1)
            emit_S(1, 0)
            emit_A(0, 3)
            emit_PV(1, 0)
            emit_PJ(0)
            emit_S(0, 2)
            emit_A(1, 2)
            emit_PV(0, 2)
            emit_S(1, 1)
            emit_PV(1, 1)
            emit_PJ(1)
            emit_S(0, 3)
            emit_A(1, 3)
            emit_PV(0, 3)
            emit_S(1, 2)
            emit_PV(1, 2)
            emit_PJ(2)
            emit_S(1, 3)
            emit_PV(1, 3)
            emit_PJ(3)

    nc.compile()
    return nc


def _get_nc(with_bias: bool):
    key = ("nc", with_bias)
    if key not in _CACHE:
        _ensure_runtime()
        _CACHE[key] = _build(with_bias)
    return _CACHE[key]


def _shard_inputs(x, w_qkv, b_qkv, w_proj, with_bias):
    """Build the 8 per-core input maps (bf16)."""
    in_maps = []
    for core in range(N_CORES):
        b, g = core // G, core % G
        hs = [g * HPG + i for i in range(HPG)]
        q_cols = [w_qkv[:, h * DH:(h + 1) * DH] for h in hs]
        k_cols = [w_qkv[:, C + h * DH: C + (h + 1) * DH] for h in hs]
        v_cols = [w_qkv[:, 2 * C + h * DH: 2 * C + (h + 1) * DH] for h in hs]
        m = {
            "xT": np.ascontiguousarray(x[b].T).astype(BF),
            "wqk": np.concatenate(q_cols + k_cols, axis=1).astype(BF),
            "wv": np.concatenate(v_cols, axis=1).astype(BF),
            "wp": np.concatenate(
                [w_proj[h * DH:(h + 1) * DH, :] for h in hs], axis=0).astype(BF),
        }
        if with_bias:
            bq = [b_qkv[h * DH:(h + 1) * DH] for h in hs]
            bk = [b_qkv[C + h * DH: C + (h + 1) * DH] for h in hs]
            bvs = [b_qkv[2 * C + h * DH: 2 * C + (h + 1) * DH] for h in hs]
            m["bqk"] = np.concatenate(bq + bk)[None, :].astype(BF)
            m["bv"] = np.concatenate(bvs)[None, :].astype(BF)
        in_maps.append(m)
    return in_maps


def run_on_device(x, w_qkv, b_qkv, w_proj, b_proj, trace=False, trace_kwargs=None):
    """Returns (output [B,T,C] float32, BassKernelResults)."""
    x = np.asarray(x, np.float32)
    w_qkv = np.asarray(w_qkv, np.float32)
    b_qkv = np.asarray(b_qkv, np.float32)
    w_proj = np.asarray(w_proj, np.float32)
    b_proj = np.asarray(b_proj, np.float32)

    with_bias = bool(np.any(b_qkv))
    nc = _get_nc(with_bias)
    in_maps = _shard_inputs(x, w_qkv, b_qkv, w_proj, with_bias)

    from concourse.bass_utils import run_bass_kernel_spmd
    res = run_bass_kernel_spmd(nc, in_maps, core_ids=list(range(N_CORES)),
                               trace=trace, **(trace_kwargs or {}))

    out = np.zeros((B, T, C), np.float64)
    for core in range(N_CORES):
        b = core // G
        out[b] += res.results[core]["outT"].T.astype(np.float64)
    out += b_proj.astype(np.float64)[None, None, :]
    return out.astype(np.float32), res


def kernel(x, w_qkv, b_qkv, w_proj, b_proj):
    out, _ = run_on_device(x, w_qkv, b_qkv, w_proj, b_proj)
    return out
